# revision 11
# baseline (speedup 1.0000x reference)
"""Trainium2 Bass kernel for nn_Attention2Conv (2-layer GCN + attention pooling).

Full inputs in, full outputs out. Internally shards the graph across 8
NeuronCores by destination node (392 node-tiles of 128 nodes, 49 tiles per
core); every core keeps a full replica of the layer-1 gather table, layer-2
table is exchanged with one AllGather, pooled graph vectors with one
AllReduce.

Math identity used (PyG GCNConv with self loops, eval-mode BN):
    out[d] = dis[d] * sum_{e: dst(e)=d} (dis[src(e)] * (h @ W)[src(e)])  + b
with dis = rsqrt(indeg + 1), self loops included as regular edges.
The src-side dis is folded into the gather table (hws = dis * (h @ W)); the
dst-side dis is applied after aggregation; bias+BN fold into one per-feature
affine fused with ReLU on the scalar engine.

Aggregation is scatter-free: edges are pre-sorted by dst block; gathered
message chunks M [128 edges, 128 feat] are combined with a one-hot matrix
S[e, j] = (dstloc[e] == j) built on the vector engine, and the tensor engine
accumulates PSUM[dst, feat] += S.T @ M per 128-node block.

dma_gather indices are int16, so the gather table is addressed through a
lo/hi split at row 32768.
"""

import numpy as np

EPS = 1e-5
NC = 8
H = 128  # feature width (F == H1 == H2 == 128)
SPLIT_DEFAULT = 32768
PAD_ID = 254.0  # dstloc / batch id for padding lanes; never matches iota 0..127


# ----------------------------------------------------------------------------
# host-side preprocessing
# ----------------------------------------------------------------------------

def _make_geom(n_nodes, n_graphs, split=SPLIT_DEFAULT):
    t_raw = -(-n_nodes // 128)
    T = -(-t_raw // NC) * NC           # node tiles, padded to multiple of NC
    NT = T * 128
    TPC = T // NC                      # tiles per core
    NPC = TPC * 128                    # nodes per core
    split = min(split, NT)
    assert split % 128 == 0 and split <= 32768 and NT - split <= 32768
    return dict(N=n_nodes, G=n_graphs, T=T, NT=NT, TPC=TPC, NPC=NPC, SPLIT=split)


def _prep_edges(src, dst, g):
    """Per-core padded gather-index / dstloc tables. Two passes: first sizes
    (global L_lo/L_hi so the SPMD program is identical on all cores), then
    layout."""
    N, NPC, TPC, SPLIT = g["N"], g["NPC"], g["TPC"], g["SPLIT"]
    per_core = []
    L_lo = L_hi = 0
    for c in range(NC):
        base = c * NPC
        sel = (dst >= base) & (dst < base + NPC)
        es = src[sel]
        ed = dst[sel]
        loops = np.arange(base, min(base + NPC, N), dtype=es.dtype)
        es = np.concatenate([es, loops])
        ed = np.concatenate([ed, loops])
        edl = ed - base
        blk = edl >> 7
        ishi = (es >= SPLIT).astype(np.int64)
        key = blk * 2 + ishi
        order = np.argsort(key, kind="stable")
        es, edl, key = es[order], edl[order], key[order]
        cnt = np.bincount(key, minlength=TPC * 2)
        lo_c = cnt[0::2].max() if TPC else 0
        hi_c = cnt[1::2].max() if TPC else 0
        L_lo = max(L_lo, -(-int(lo_c) // 128))
        L_hi = max(L_hi, -(-int(hi_c) // 128))
        per_core.append((es, edl, key, cnt))

    L_lo = max(L_lo, 1)
    CHB = L_lo + L_hi
    tot = TPC * CHB * 128
    maps = []
    for es, edl, key, cnt in per_core:
        starts = np.concatenate([[0], np.cumsum(cnt)])
        within = np.arange(len(es)) - starts[key]
        blk2 = key >> 1
        hi2 = key & 1
        dest = blk2 * (CHB * 128) + hi2 * (L_lo * 128) + within
        idx_arr = np.zeros(tot, np.int16)
        dl_arr = np.full(tot, PAD_ID, np.float32)
        idx_arr[dest] = (es - hi2 * SPLIT).astype(np.int16)
        dl_arr[dest] = (edl & 127).astype(np.float32)
        idx_wrapped = np.ascontiguousarray(
            np.tile(idx_arr.reshape(-1, 16).T, (8, 1))
        )  # [128, tot//16]
        dstloc = np.ascontiguousarray(dl_arr.reshape(-1, 128).T)  # [128, nchunks]
        maps.append((idx_wrapped, dstloc))
    return maps, L_lo, L_hi


def _prepare(inputs, split=SPLIT_DEFAULT, n_graphs=64):
    x = np.asarray(inputs["x"], np.float32)
    edge_index = np.asarray(inputs["edge_index"])
    batch = np.asarray(inputs["batch"])
    N = x.shape[0]
    g = _make_geom(N, n_graphs, split)
    NT, TPC, NPC, T = g["NT"], g["TPC"], g["NPC"], g["T"]

    src = edge_index[0].astype(np.int64)
    dst = edge_index[1].astype(np.int64)

    deg = np.bincount(dst, minlength=N).astype(np.float32) + 1.0
    deg_pad = np.ones(NT, np.float32)
    deg_pad[:N] = deg

    bat_pad = np.full(NT, PAD_ID, np.float32)
    bat_pad[:N] = batch.astype(np.float32)

    xT = np.zeros((128, NT), np.float32)
    xT[:, :N] = np.asarray(x, np.float32).T

    def aff(gam, bet, mean, var, b):
        A = np.asarray(gam, np.float32) / np.sqrt(np.asarray(var, np.float32) + EPS)
        C = (np.asarray(b, np.float32) - np.asarray(mean, np.float32)) * A + np.asarray(bet, np.float32)
        return A.reshape(128, 1), C.reshape(128, 1)

    A1, C1 = aff(inputs["bn1_gamma"], inputs["bn1_beta"], inputs["bn1_mean"],
                 inputs["bn1_var"], inputs["b1"])
    A2, C2 = aff(inputs["bn2_gamma"], inputs["bn2_beta"], inputs["bn2_mean"],
                 inputs["bn2_var"], inputs["b2"])

    edge_maps, L_lo, L_hi = _prep_edges(src, dst, g)
    g["L_lo"], g["L_hi"] = L_lo, L_hi

    iota = np.ascontiguousarray(
        np.broadcast_to(np.arange(128, dtype=np.float32), (128, 128)))
    ident = np.eye(128, dtype=np.float32)

    common = {
        "xT_in": xT,
        "W1_in": np.asarray(inputs["W1"], np.float32),
        "W2_in": np.asarray(inputs["W2"], np.float32),
        "attw_in": np.asarray(inputs["att_w"], np.float32).reshape(128, 1),
        "clsrep_in": np.ascontiguousarray(np.broadcast_to(
            np.asarray(inputs["cls_w"], np.float32).reshape(1, 128), (128, 128))),
        "clsbrep_in": np.full((128, 1), np.asarray(inputs["cls_b"], np.float32).reshape(()) / NC, np.float32),
        "A1_in": A1, "C1_in": C1, "A2_in": A2, "C2_in": C2,
        "degg_in": np.ascontiguousarray(deg_pad.reshape(T, 128).T),
        "iota_in": iota,
        "ident_in": ident,
    }
    in_maps = []
    for c in range(NC):
        idx_wrapped, dstloc = edge_maps[c]
        m = dict(common)
        m["idx_in"] = idx_wrapped
        m["dstloc_in"] = dstloc
        m["dego_in"] = np.ascontiguousarray(
            deg_pad[c * NPC:(c + 1) * NPC].reshape(TPC, 128).T)
        m["batch_in"] = np.ascontiguousarray(
            bat_pad[c * NPC:(c + 1) * NPC].reshape(TPC, 128).T)
        in_maps.append(m)
    return g, in_maps


# ----------------------------------------------------------------------------
# device program
# ----------------------------------------------------------------------------

STOP_AFTER = "E"  # debug knob: truncate program after phase A/B/C/D/E


def _build_program(g):
    import concourse.bacc as bacc
    import concourse.mybir as mybir
    import concourse.tile as tile
    from concourse import library_config
    stage = "ABCDE".index(STOP_AFTER)

    T, NT, TPC, NPC, SPLIT = g["T"], g["NT"], g["TPC"], g["NPC"], g["SPLIT"]
    L_lo, L_hi, G = g["L_lo"], g["L_hi"], g["G"]
    CHB = L_lo + L_hi
    NCH = TPC * CHB
    IDXC = NCH * 8  # idx columns (128 idx/chunk / 16 per col)
    f32 = mybir.dt.float32
    AF = mybir.ActivationFunctionType
    ALU = mybir.AluOpType

    nc = bacc.Bacc("TRN2", target_bir_lowering=False, debug=False, num_devices=NC)

    # inputs
    xT_in = nc.dram_tensor("xT_in", [128, NT], f32, kind="ExternalInput")
    W1_in = nc.dram_tensor("W1_in", [128, 128], f32, kind="ExternalInput")
    W2_in = nc.dram_tensor("W2_in", [128, 128], f32, kind="ExternalInput")
    attw_in = nc.dram_tensor("attw_in", [128, 1], f32, kind="ExternalInput")
    clsrep_in = nc.dram_tensor("clsrep_in", [128, 128], f32, kind="ExternalInput")
    clsbrep_in = nc.dram_tensor("clsbrep_in", [128, 1], f32, kind="ExternalInput")
    A1_in = nc.dram_tensor("A1_in", [128, 1], f32, kind="ExternalInput")
    C1_in = nc.dram_tensor("C1_in", [128, 1], f32, kind="ExternalInput")
    A2_in = nc.dram_tensor("A2_in", [128, 1], f32, kind="ExternalInput")
    C2_in = nc.dram_tensor("C2_in", [128, 1], f32, kind="ExternalInput")
    degg_in = nc.dram_tensor("degg_in", [128, T], f32, kind="ExternalInput")
    dego_in = nc.dram_tensor("dego_in", [128, TPC], f32, kind="ExternalInput")
    iota_in = nc.dram_tensor("iota_in", [128, 128], f32, kind="ExternalInput")
    ident_in = nc.dram_tensor("ident_in", [128, 128], f32, kind="ExternalInput")
    idx_in = nc.dram_tensor("idx_in", [128, IDXC], mybir.dt.int16, kind="ExternalInput")
    dstloc_in = nc.dram_tensor("dstloc_in", [128, NCH], f32, kind="ExternalInput")
    batch_in = nc.dram_tensor("batch_in", [128, TPC], f32, kind="ExternalInput")

    # outputs
    att_out = nc.dram_tensor("att_out", [NPC, 1], f32, kind="ExternalOutput")
    logits_out = nc.dram_tensor("logits_out", [G, 1], f32, kind="ExternalOutput")

    # internal DRAM
    hws1_tab = nc.dram_tensor("hws1_tab", [NT, 128], f32, kind="Internal")
    hws2_own = nc.dram_tensor("hws2_own", [NPC, 128], f32, kind="Internal")
    hws2_tab = nc.dram_tensor("hws2_tab", [NT, 128], f32, kind="Internal",
                              addr_space="Shared")
    g_part = nc.dram_tensor("g_part", [G, 128], f32, kind="Internal")
    g_full = nc.dram_tensor("g_full", [G, 128], f32, kind="Internal",
                            addr_space="Shared")

    hws1_t = hws1_tab.ap().rearrange("(t p) h -> p t h", p=128)
    hws2o_t = hws2_own.ap().rearrange("(t p) h -> p t h", p=128)
    att_t = att_out.ap().rearrange("(t p) one -> p t one", p=128)

    XG = 4
    while T % XG:
        XG -= 1

    with tile.TileContext(nc) as tc:
        nc.gpsimd.load_library(library_config.mlp)
        with (
            tc.tile_pool(name="const", bufs=1) as constp,
            tc.tile_pool(name="persist", bufs=1) as persist,
        ):
            # ---- constants to SBUF
            def cload(name, src_ap, shape, dtype=f32):
                t = constp.tile(shape, dtype, tag=name)
                nc.sync.dma_start(out=t[...], in_=src_ap)
                return t

            W1_sb = cload("W1", W1_in.ap(), [128, 128])
            W2_sb = cload("W2", W2_in.ap(), [128, 128])
            attw_sb = cload("attw", attw_in.ap(), [128, 1])
            clsrep_sb = cload("clsrep", clsrep_in.ap(), [128, 128])
            clsbrep_sb = cload("clsbrep", clsbrep_in.ap(), [128, 1])
            A1_sb = cload("A1", A1_in.ap(), [128, 1])
            C1_sb = cload("C1", C1_in.ap(), [128, 1])
            A2_sb = cload("A2", A2_in.ap(), [128, 1])
            C2_sb = cload("C2", C2_in.ap(), [128, 1])
            iota_sb = cload("iota", iota_in.ap(), [128, 128])
            ident_sb = cload("ident", ident_in.ap(), [128, 128])
            batch_sb = cload("batch", batch_in.ap(), [128, TPC])
            idx_sb = cload("idx", idx_in.ap(), [128, IDXC], mybir.dt.int16)
            dstloc_sb = cload("dstloc", dstloc_in.ap(), [128, NCH])

            degg_sb = cload("degg", degg_in.ap(), [128, T])
            dego_sb = cload("dego", dego_in.ap(), [128, TPC])
            # dis = sqrt(1/deg)
            disg_sb = persist.tile([128, T], f32, tag="disg")
            nc.vector.reciprocal(disg_sb[:, :], degg_sb[:, :])
            nc.scalar.sqrt(disg_sb[:, :], disg_sb[:, :])
            diso_sb = persist.tile([128, TPC], f32, tag="diso")
            nc.vector.reciprocal(diso_sb[:, :], dego_sb[:, :])
            nc.scalar.sqrt(diso_sb[:, :], diso_sb[:, :])

            h1T_sb = persist.tile([128, TPC, 128], f32, tag="h1T")
            g_acc = persist.tile([G, 128], f32, tag="gacc")
            nc.vector.memset(g_acc[:, :], 0.0)

            # ---- phase A: hws1[n, :] = dis[n] * (x @ W1)[n, :], all nodes
            with (
                tc.tile_pool(name="pA", bufs=3) as pA,
                tc.tile_pool(name="psA", bufs=3, space="PSUM") as psA,
            ):
                for gi in range(T // XG):
                    xt = pA.tile([128, XG, 128], f32, tag="xt")
                    nc.sync.dma_start(
                        out=xt[...],
                        in_=xT_in.ap()[:, gi * XG * 128:(gi + 1) * XG * 128]
                            .rearrange("p (g n) -> p g n", g=XG))
                    hv = pA.tile([128, XG, 128], f32, tag="hv")
                    for j in range(XG):
                        t_idx = gi * XG + j
                        ps = psA.tile([128, 128], f32, tag="psA")
                        nc.tensor.matmul(ps[:, :], xt[:, j, :], W1_sb[:, :],
                                         start=True, stop=True)
                        nc.vector.tensor_scalar_mul(
                            hv[:, j, :], ps[:, :], disg_sb[:, t_idx:t_idx + 1])
                    nc.sync.dma_start(
                        out=hws1_t[:, gi * XG:(gi + 1) * XG, :], in_=hv[...])

            # ---- phase B/D shared block body
            def agg_layer(tab_ap, out_cb):
                """For each owned block: gather + one-hot matmul aggregation;
                out_cb(b, ps_agg) consumes the raw aggregated PSUM tile."""
                with (
                    tc.tile_pool(name="msg", bufs=2) as msgp,
                    tc.tile_pool(name="Sp", bufs=2) as Sp,
                    tc.tile_pool(name="psB", bufs=2, space="PSUM") as psB,
                ):
                    for b in range(TPC):
                        cb = b * CHB
                        parts = [(0, L_lo, tab_ap)]
                        if L_hi:
                            parts.append((L_lo, L_hi, tab_ap[SPLIT:, :]))
                        ps = psB.tile([128, 128], f32, tag="agg")
                        first = True
                        for (coff, ln, src_ap) in parts:
                            msg = msgp.tile([128, ln, 128], f32, tag=f"m{coff>0}")
                            # dma_gather is limited to 1024 idxs per call (64
                            # descriptors x 16 SDMA engines in one packet)
                            for s0 in range(0, ln, 8):
                                sl = min(8, ln - s0)
                                nc.gpsimd.dma_gather(
                                    msg[:, s0:s0 + sl, :], src_ap,
                                    idx_sb[:, (cb + coff + s0) * 8:
                                           (cb + coff + s0 + sl) * 8],
                                    sl * 128, sl * 128, 128)
                            S = Sp.tile([128, ln, 128], f32, tag=f"S{coff>0}")
                            nc.vector.tensor_tensor(
                                out=S[...],
                                in0=iota_sb[:, None, :].broadcast_to([128, ln, 128]),
                                in1=dstloc_sb[:, cb + coff:cb + coff + ln, None]
                                    .broadcast_to([128, ln, 128]),
                                op=ALU.is_equal)
                            for ci in range(ln):
                                last = (coff + ci == CHB - 1)
                                nc.tensor.matmul(ps[:, :], S[:, ci, :],
                                                 msg[:, ci, :],
                                                 start=first, stop=last)
                                first = False
                        out_cb(b, ps)

            # ---- phase B: layer-1 aggregation -> h1T (SBUF resident)
            def phase_b():
                with (
                    tc.tile_pool(name="epi1", bufs=3) as epi1,
                    tc.tile_pool(name="psT1", bufs=2, space="PSUM") as psT1,
                ):
                    def l1_out(b, ps):
                        v = epi1.tile([128, 128], f32, tag="v1")
                        nc.vector.tensor_scalar_mul(v[:, :], ps[:, :],
                                                    diso_sb[:, b:b + 1])
                        pst = psT1.tile([128, 128], f32, tag="pst1")
                        nc.tensor.transpose(pst[:, :], v[:, :], ident_sb[:, :])
                        nc.scalar.activation(h1T_sb[:, b, :], pst[:, :], AF.Relu,
                                             bias=C1_sb[:, 0:1], scale=A1_sb[:, 0:1])
                    agg_layer(hws1_tab.ap(), l1_out)

            # ---- phase C: hws2_own = dis * (h1 @ W2); AllGather
            def phase_c():
                with (
                    tc.tile_pool(name="pC", bufs=3) as pC,
                    tc.tile_pool(name="psC", bufs=3, space="PSUM") as psC,
                ):
                    for b in range(TPC):
                        ps = psC.tile([128, 128], f32, tag="psC")
                        nc.tensor.matmul(ps[:, :], h1T_sb[:, b, :], W2_sb[:, :],
                                         start=True, stop=True)
                        u = pC.tile([128, 128], f32, tag="u")
                        nc.vector.tensor_scalar_mul(u[:, :], ps[:, :],
                                                    diso_sb[:, b:b + 1])
                        nc.sync.dma_start(out=hws2o_t[:, b, :], in_=u[:, :])
                nc.gpsimd.collective_compute(
                    "AllGather", mybir.AluOpType.bypass,
                    replica_groups=[list(range(NC))],
                    ins=[hws2_own.ap()], outs=[hws2_tab.ap()])

            # ---- phase D: layer-2 aggregation -> att, pooling
            def phase_d():
                with (
                    tc.tile_pool(name="epi2", bufs=3) as epi2,
                    tc.tile_pool(name="psT2", bufs=1, space="PSUM") as psT2,
                    tc.tile_pool(name="psE2", bufs=1, space="PSUM") as psE2,
                ):
                    def l2_out(b, ps):
                        v = epi2.tile([128, 128], f32, tag="v2")
                        nc.vector.tensor_scalar_mul(v[:, :], ps[:, :],
                                                    diso_sb[:, b:b + 1])
                        pst = psT2.tile([128, 128], f32, tag="pst2")
                        nc.tensor.transpose(pst[:, :], v[:, :], ident_sb[:, :])
                        h2T = epi2.tile([128, 128], f32, tag="h2T")
                        nc.scalar.activation(h2T[:, :], pst[:, :], AF.Relu,
                                             bias=C2_sb[:, 0:1], scale=A2_sb[:, 0:1])
                        pa = psE2.tile([128, 1], f32, tag="pa")
                        nc.tensor.matmul(pa[:, :], h2T[:, :], attw_sb[:, :],
                                         start=True, stop=True)
                        att_sb = epi2.tile([128, 1], f32, tag="att")
                        nc.scalar.activation(att_sb[:, :], pa[:, :], AF.Sigmoid)
                        nc.sync.dma_start(out=att_t[:, b, :], in_=att_sb[:, :])
                        ph = psT2.tile([128, 128], f32, tag="ph")
                        nc.tensor.transpose(ph[:, :], h2T[:, :], ident_sb[:, :])
                        m2 = epi2.tile([128, 128], f32, tag="m2")
                        nc.vector.tensor_scalar_mul(m2[:, :], ph[:, :],
                                                    att_sb[:, 0:1])
                        Bb = epi2.tile([128, G], f32, tag="Bb")
                        nc.vector.tensor_scalar(
                            out=Bb[:, :], in0=iota_sb[:, :G],
                            scalar1=batch_sb[:, b:b + 1], scalar2=None,
                            op0=ALU.is_equal)
                        pg = psE2.tile([G, 128], f32, tag="pg")
                        nc.tensor.matmul(pg[:, :], Bb[:, :], m2[:, :],
                                         start=True, stop=True)
                        nc.vector.tensor_tensor(out=g_acc[:, :], in0=g_acc[:, :],
                                                in1=pg[:, :], op=ALU.add)
                    agg_layer(hws2_tab.ap(), l2_out)

            # ---- phase E: partial logits per core (pre-collective, since
            # AllReduce is linear), then AllReduce + plain DMAs only (PE or
            # reduce work after a collective deadlocks/crashes on HW here)
            def phase_e():
                with (
                    tc.tile_pool(name="pE", bufs=1) as pE,
                ):
                    tmp = pE.tile([G, 128], f32, tag="tmpE")
                    nc.vector.tensor_tensor(out=tmp[:, :], in0=g_acc[:, :],
                                            in1=clsrep_sb[:G, :], op=ALU.mult)
                    tmp2 = pE.tile([G, 128], f32, tag="tmpE2")
                    lgp = pE.tile([G, 1], f32, tag="lgp")
                    nc.scalar.activation(tmp2[:, :], tmp[:, :], AF.Copy,
                                         accum_out=lgp[:, :])
                    nc.vector.tensor_scalar_add(lgp[:, :], lgp[:, :],
                                                clsbrep_sb[:G, 0:1])
                    lpad = pE.tile([G, 128], f32, tag="lpad")
                    nc.vector.memset(lpad[:, :], 0.0)
                    nc.vector.tensor_copy(lpad[:, 0:1], lgp[:, :])
                    nc.sync.dma_start(out=g_part.ap(), in_=lpad[:, :])
                    nc.gpsimd.collective_compute(
                        "AllReduce", mybir.AluOpType.add,
                        replica_groups=[list(range(NC))],
                        ins=[g_part.ap()], outs=[g_full.ap()])
                    gf = pE.tile([G, 128], f32, tag="gf")
                    nc.sync.dma_start(out=gf[:, :], in_=g_full.ap())
                    nc.sync.dma_start(out=logits_out.ap(), in_=gf[:, 0:1])

            for lvl, fn in enumerate([phase_b, phase_c, phase_d, phase_e], start=1):
                if stage >= lvl:
                    fn()

    nc.compile()
    return nc


# ----------------------------------------------------------------------------
# entry point
# ----------------------------------------------------------------------------

_CACHE = {}


def _get_program(g):
    key = (g["T"], g["NT"], g["SPLIT"], g["L_lo"], g["L_hi"], g["G"])
    if key not in _CACHE:
        _CACHE[key] = _build_program(g)
    return _CACHE[key]


def run(inputs, split=SPLIT_DEFAULT, n_graphs=64, trace=False):
    """Run on 8 cores; returns (logits [G,1], att [N,1], exec_time_ns)."""
    from concourse.bass_utils import run_bass_kernel_spmd

    g, in_maps = _prepare(inputs, split, n_graphs)
    nc = _get_program(g)
    res = run_bass_kernel_spmd(nc, in_maps, core_ids=list(range(NC)),
                               trace=trace)
    att = np.concatenate([res.results[c]["att_out"] for c in range(NC)],
                         axis=0)[:g["N"]]
    logits = res.results[0]["logits_out"].reshape(g["G"], 1)
    return logits, att, res.exec_time_ns


def kernel(**inputs):
    logits, att, _ = run(inputs)
    return logits, att
